# revision 20
# baseline (speedup 1.0000x reference)
"""NodeContrastiveLoss Trainium2 kernel.

Full inputs -> scalar loss, data-parallel over 8 NeuronCores (256 batches/core).

Per batch b (reference semantics):
  sums[f,d]  = segment-sum of atom_embed over atom2frag
  mn         = sums / max(||sums||, eps)        (== means/||means|| since count scale cancels)
  fn         = frag / max(||frag||, eps)        (host-precomputed)
  sims       = 10 * mn @ fn.T
  per_frag   = logsumexp(sims, -1) - diag(sims)
  loss       = sum(valid * per_frag) / max(n_valid, 1)

Device pipeline per 8-batch iteration (4 pairs of 2 batches stacked on 128 partitions):
  gpsimd : one-hot = (iota == idx) as bf16
  PE     : 16 seg-matmuls (one-hot.T @ atoms) -> PSUM sums [128,4,128]
  ACT    : copy sums -> SBUF bf16
  DVE    : TTR ssq, clamp; build D = diag(10*rsqrt(ssq)) via ident*scale
  ACT    : ln/exp small ops for the scale (sqrt avoided: stays in one ACT table set)
  PE     : 4 transpose-matmuls sums.T @ D -> scaled mnT (fused transpose+normalize)
  ACT    : copy mnT -> SBUF
  PE     : 4 sims matmuls mnT.T @ fragT_hat -> PSUM [128,4,128]
  ACT    : exp(sims) -> SBUF bf16
  DVE    : TTR x8 (halfmask -> row sums s, identity -> exp(pos)); per_frag = ln(s/spos)
  DVE    : masked per-frag -> accumulation slab; final reduce -> out[128,1]
Host: sum 8x128 partials, divide by n_valid.
"""

import sys

sys.path.insert(0, "/opt/trn_rl_repo")

from contextlib import ExitStack

import ml_dtypes
import numpy as np

import concourse.bacc as bacc
import concourse.bass as bass
import concourse.tile as tile
from concourse import mybir
from concourse.bass_utils import run_bass_kernel_spmd

B, A, F_, D = 2048, 256, 64, 128
NCORES = 8
BPC = B // NCORES          # 256 batches per core
BPI = 8                    # batches per iteration
ITERS = BPC // BPI         # 32
PAIRS = BPI // 2           # 4
CHUNKS = BPI * 2           # 16 chunks of 128 atoms

BF16 = mybir.dt.bfloat16
F32 = mybir.dt.float32
U8 = mybir.dt.uint8
I32 = mybir.dt.int32
ALU = mybir.AluOpType
ACTF = mybir.ActivationFunctionType
AXIS = mybir.AxisListType

LN10 = float(np.log(10.0))


def build_body(tc, outs, ins):
    """Tile kernel body. ins/outs: dicts of DRAM APs.

    ins: atoms [ITERS,128,CHUNKS*128] bf16, fragT [ITERS,128,PAIRS*128] bf16,
         idx [ITERS,128,CHUNKS] u8, mask [ITERS,128,PAIRS] bf16
    outs: out [128,1] f32
    """
    nc = tc.nc
    ctx = ExitStack()
    with ctx:
        const = ctx.enter_context(tc.tile_pool(name="const", bufs=1))
        dpool = ctx.enter_context(tc.tile_pool(name="dma", bufs=4))
        work = ctx.enter_context(tc.tile_pool(name="work", bufs=3))
        small = ctx.enter_context(tc.tile_pool(name="small", bufs=3))
        pseg = ctx.enter_context(tc.tile_pool(name="pseg", bufs=2, space="PSUM"))
        ptr = ctx.enter_context(tc.tile_pool(name="ptr", bufs=2, space="PSUM"))
        psim = ctx.enter_context(tc.tile_pool(name="psim", bufs=2, space="PSUM"))

        # ---- one-time constants ----
        iota_i32 = const.tile([128, CHUNKS, 64], I32)
        nc.gpsimd.iota(iota_i32[:], [[0, CHUNKS], [1, 64]], channel_multiplier=0)
        iota_bf = const.tile([128, CHUNKS, 64], BF16)
        nc.gpsimd.tensor_copy(iota_bf[:], iota_i32[:])

        # identity (bf16): ident[p,f] = (p == f)
        irow = const.tile([128, 128], I32)
        nc.gpsimd.iota(irow[:], [[1, 128]], channel_multiplier=0)
        icol = const.tile([128, 1], I32)
        nc.gpsimd.iota(icol[:], [[0, 1]], channel_multiplier=1)
        irow_f = const.tile([128, 128], F32)
        nc.gpsimd.tensor_copy(irow_f[:], irow[:])
        icol_f = const.tile([128, 1], F32)
        nc.gpsimd.tensor_copy(icol_f[:], icol[:])
        ident = const.tile([128, 128], BF16)
        nc.vector.tensor_scalar(ident[:], irow_f[:], icol_f[:], None, op0=ALU.is_equal)

        # accumulation slab: [128, ITERS, PAIRS] f32, fully written each run
        slab = const.tile([128, ITERS, PAIRS], F32)

        ln10_t = const.tile([128, 1], F32)
        nc.gpsimd.memset(ln10_t[:], LN10)

        for it in range(ITERS):
            # ---- DMA in (split for queue parallelism) ----
            atoms_t = dpool.tile([128, CHUNKS, 128], BF16, tag="atoms")
            for s in range(8):
                nc.sync.dma_start(
                    atoms_t[:, 2 * s : 2 * s + 2, :],
                    ins["atoms"][it, :, 256 * s : 256 * (s + 1)],
                )
            fragT_t = dpool.tile([128, PAIRS, 128], BF16, tag="fragT")
            for s in range(2):
                nc.sync.dma_start(
                    fragT_t[:, 2 * s : 2 * s + 2, :],
                    ins["fragT"][it, :, 256 * s : 256 * (s + 1)],
                )
            idx_t = dpool.tile([128, CHUNKS], U8, tag="idx")
            nc.sync.dma_start(idx_t[:], ins["idx"][it])
            mask_t = dpool.tile([128, PAIRS], BF16, tag="mask")
            nc.sync.dma_start(mask_t[:], ins["mask"][it])

            # ---- one-hot on gpsimd ----
            idx_bf = small.tile([128, CHUNKS], BF16, tag="idxbf")
            nc.gpsimd.tensor_copy(idx_bf[:], idx_t[:])
            onehot = work.tile([128, CHUNKS, 64], BF16, tag="onehot")
            idx_bc = idx_bf[:].unsqueeze(2).broadcast_to((128, CHUNKS, 64))
            nc.vector.tensor_tensor(onehot[:], iota_bf[:], idx_bc, op=ALU.is_equal)

            # ---- segment-sum matmuls: psum_seg[64*tw:+64, j, :] += onehot_c.T @ atoms_c
            psum_seg = pseg.tile([128, PAIRS, 128], F32, tag="seg")
            for c in range(CHUNKS):
                bi, h = divmod(c, 2)
                j, tw = divmod(bi, 2)
                nc.tensor.matmul(
                    psum_seg[64 * tw : 64 * tw + 64, j, :],
                    onehot[:, c, :],
                    atoms_t[:, c, :],
                    start=(h == 0),
                    stop=(h == 1),
                )

            # ---- sums -> SBUF bf16 (ACT copy) ----
            sums_s = work.tile([128, PAIRS, 128], BF16, tag="sums")
            nc.scalar.copy(sums_s[:], psum_seg[:])

            # ---- ssq + normalization scale: scale = 10 / max(||sums||, 1e-8)
            sq_s = work.tile([128, PAIRS, 128], BF16, tag="sqs")
            nc.vector.tensor_tensor(sq_s[:], sums_s[:], sums_s[:], op=ALU.mult)
            ssq = small.tile([128, PAIRS], F32, tag="ssq")
            nc.vector.tensor_reduce(ssq[:], sq_s[:], axis=AXIS.X, op=ALU.add)
            ssq_c = small.tile([128, PAIRS], F32, tag="ssqc")
            nc.vector.tensor_scalar(ssq_c[:], ssq[:], 1e-16, None, op0=ALU.max)
            lnssq = small.tile([128, PAIRS], F32, tag="lnssq")
            nc.scalar.activation(lnssq[:], ssq_c[:], ACTF.Ln)
            scale_mn = small.tile([128, PAIRS], F32, tag="scale")
            # exp(-0.5*ln(ssq) + ln(10)) = 10 * rsqrt(ssq)
            nc.scalar.activation(
                scale_mn[:], lnssq[:], ACTF.Exp, bias=ln10_t[:], scale=-0.5
            )

            # ---- D = diag(scale): ident row p has its single 1 at col p ----
            D_t = work.tile([128, PAIRS, 128], BF16, tag="D")
            for j in range(PAIRS):
                nc.vector.tensor_scalar(
                    D_t[:, j, :], ident[:], scale_mn[:, j : j + 1], None, op0=ALU.mult
                )

            # ---- fused transpose+normalize (regular matmul): mnT = sums.T @ D
            psum_tr = ptr.tile([128, PAIRS, 128], F32, tag="tr")
            for j in range(PAIRS):
                nc.tensor.matmul(
                    psum_tr[:, j, :],
                    sums_s[:, j, :],
                    D_t[:, j, :],
                    start=True,
                    stop=True,
                )
            mnT_s = work.tile([128, PAIRS, 128], BF16, tag="mnT")
            nc.scalar.copy(mnT_s[:], psum_tr[:])

            # ---- sims matmuls (rows already normalized & x10) ----
            psum_sims = psim.tile([128, PAIRS, 128], F32, tag="sims")
            for j in range(PAIRS):
                nc.tensor.matmul(
                    psum_sims[:, j, :],
                    mnT_s[:, j, :],
                    fragT_t[:, j, :],
                    start=True,
                    stop=True,
                )

            # ---- exp over all sims (one plain ACT op) ----
            exp_s = work.tile([128, PAIRS, 128], BF16, tag="exps")
            nc.scalar.activation(exp_s[:], psum_sims[:], ACTF.Exp)

            # ---- exp(pos) = diag(exp_s) via strided DMA (off compute engines) ----
            epos = small.tile([128, PAIRS], BF16, tag="epos")
            e_h = exp_s[:, 0, 0]
            diag_ap = bass.AP(
                tensor=e_h.tensor,
                offset=e_h.offset,
                ap=[[PAIRS * 128 + 1, 128], [128, PAIRS]],
            )
            nc.sync.dma_start(epos[:], diag_ap)

            # ---- per-half row sums, select own half ----
            s_half = small.tile([128, PAIRS, 2], F32, tag="shalf")
            exp_v = exp_s[:].rearrange("p j (two g) -> p j two g", two=2)
            nc.vector.tensor_reduce(s_half[:], exp_v, axis=AXIS.X, op=ALU.add)
            s_sel = small.tile([128, PAIRS], F32, tag="ssel")
            nc.vector.tensor_copy(s_sel[0:64, :], s_half[0:64, :, 0])
            nc.vector.tensor_copy(s_sel[64:128, :], s_half[64:128, :, 1])

            # ---- per_frag = ln(s_sel / exp(pos)); masked into slab ----
            inv_ep = small.tile([128, PAIRS], F32, tag="invep")
            nc.vector.reciprocal(inv_ep[:], epos[:])
            ratio = small.tile([128, PAIRS], F32, tag="ratio")
            nc.vector.tensor_tensor(ratio[:], s_sel[:], inv_ep[:], op=ALU.mult)
            perfrag = small.tile([128, PAIRS], F32, tag="perfrag")
            nc.scalar.activation(perfrag[:], ratio[:], ACTF.Ln)
            nc.vector.tensor_tensor(
                slab[:, it, :], perfrag[:], mask_t[:], op=ALU.mult
            )

        # ---- final reduce + output ----
        outsb = const.tile([128, 1], F32)
        nc.vector.tensor_reduce(outsb[:], slab[:], axis=AXIS.XY, op=ALU.add)
        nc.sync.dma_start(outs["out"], outsb[:])


def prep_inputs(atom_embed, fragment_embed, atom2frag):
    """Host-side layout prep. Returns (in_maps, n_valid)."""
    bf = ml_dtypes.bfloat16
    am = np.asarray(atom_embed, dtype=np.float32)
    fe = np.asarray(fragment_embed, dtype=np.float32)
    af = np.asarray(atom2frag)

    # atoms: [B,A,D] -> [core, it, p, (bi,h), d]
    a6 = am.reshape(NCORES, ITERS, BPI, 2, 128, D)
    atoms_np = np.ascontiguousarray(a6.transpose(0, 1, 4, 2, 3, 5)).reshape(
        NCORES, ITERS, 128, CHUNKS * 128
    ).astype(bf)

    # frag normalized + transposed: [core, it, d, (j, tw*64+g)]
    fen = fe / np.maximum(np.linalg.norm(fe, axis=-1, keepdims=True), 1e-8)
    f6 = fen.reshape(NCORES, ITERS, PAIRS, 2, F_, D)
    fragT_np = np.ascontiguousarray(f6.transpose(0, 1, 5, 2, 3, 4)).reshape(
        NCORES, ITERS, 128, PAIRS * 128
    ).astype(bf)

    # idx: [core, it, p, (bi,h)] u8
    i5 = af.reshape(NCORES, ITERS, BPI, 2, 128)
    idx_np = np.ascontiguousarray(i5.transpose(0, 1, 4, 2, 3)).reshape(
        NCORES, ITERS, 128, CHUNKS
    ).astype(np.uint8)

    # counts/valid on host (index metadata)
    counts = (af[:, :, None] == np.arange(F_)[None, None, :]).sum(axis=1)
    valid = counts > 0
    n_valid = int(valid.sum())
    v6 = valid.reshape(NCORES, ITERS, PAIRS, 2, F_)
    mask_np = np.ascontiguousarray(v6.transpose(0, 1, 3, 4, 2)).reshape(
        NCORES, ITERS, 128, PAIRS
    ).astype(bf)

    in_maps = [
        {
            "atoms": atoms_np[k],
            "fragT": fragT_np[k],
            "idx": idx_np[k],
            "mask": mask_np[k],
        }
        for k in range(NCORES)
    ]
    return in_maps, n_valid


_BUILT = None


def build_nc():
    global _BUILT
    if _BUILT is not None:
        return _BUILT
    nc = bacc.Bacc("TRN2", target_bir_lowering=False, debug=False)
    ins = {
        "atoms": nc.dram_tensor(
            "atoms", [ITERS, 128, CHUNKS * 128], BF16, kind="ExternalInput"
        ).ap(),
        "fragT": nc.dram_tensor(
            "fragT", [ITERS, 128, PAIRS * 128], BF16, kind="ExternalInput"
        ).ap(),
        "idx": nc.dram_tensor(
            "idx", [ITERS, 128, CHUNKS], U8, kind="ExternalInput"
        ).ap(),
        "mask": nc.dram_tensor(
            "mask", [ITERS, 128, PAIRS], BF16, kind="ExternalInput"
        ).ap(),
    }
    outs = {"out": nc.dram_tensor("out", [128, 1], F32, kind="ExternalOutput").ap()}
    with tile.TileContext(nc) as tc:
        build_body(tc, outs, ins)
    nc.compile()
    _BUILT = nc
    return nc


def run_on_hw(in_maps, trace=False, **kw):
    nc = build_nc()
    return run_bass_kernel_spmd(nc, in_maps, list(range(NCORES)), trace=trace, **kw)


def kernel(**inputs) -> np.ndarray:
    in_maps, n_valid = prep_inputs(
        inputs["atom_embed"], inputs["fragment_embed"], inputs["atom2frag"]
    )
    res = run_on_hw(in_maps)
    total = 0.0
    for k in range(NCORES):
        total += float(np.asarray(res.results[k]["out"], dtype=np.float64).sum())
    if n_valid > 0:
        loss = np.float32(total / n_valid)
    else:
        loss = np.float32(0.0)
    return np.array(loss, dtype=np.float32)


# revision 27
# speedup vs baseline: 1.3952x; 1.3952x over previous
"""NodeContrastiveLoss Trainium2 kernel.

Full inputs -> scalar loss, data-parallel over 8 NeuronCores (256 batches/core).

Per batch b (reference semantics):
  sums[f,d]  = segment-sum of atom_embed over atom2frag
  mn         = sums / max(||sums||, eps)        (== means/||means|| since count scale cancels)
  fn         = frag / max(||frag||, eps)        (host-precomputed)
  sims       = 10 * mn @ fn.T
  per_frag   = logsumexp(sims, -1) - diag(sims)
  loss       = sum(valid * per_frag) / max(n_valid, 1)

Device pipeline per 8-batch iteration (4 pairs of 2 batches stacked on 128 partitions):
  gpsimd : one-hot = (iota == idx) as bf16
  PE     : 16 seg-matmuls (one-hot.T @ atoms) -> PSUM sums [128,4,128]
  ACT    : copy sums -> SBUF bf16
  DVE    : TTR ssq, clamp; build D = diag(10*rsqrt(ssq)) via ident*scale
  ACT    : ln/exp small ops for the scale (sqrt avoided: stays in one ACT table set)
  PE     : 4 transpose-matmuls sums.T @ D -> scaled mnT (fused transpose+normalize)
  ACT    : copy mnT -> SBUF
  PE     : 4 sims matmuls mnT.T @ fragT_hat -> PSUM [128,4,128]
  ACT    : exp(sims) -> SBUF bf16
  DVE    : TTR x8 (halfmask -> row sums s, identity -> exp(pos)); per_frag = ln(s/spos)
  DVE    : masked per-frag -> accumulation slab; final reduce -> out[128,1]
Host: sum 8x128 partials, divide by n_valid.
"""

import sys

sys.path.insert(0, "/opt/trn_rl_repo")

from contextlib import ExitStack

import ml_dtypes
import numpy as np

import concourse.bacc as bacc
import concourse.bass as bass
import concourse.tile as tile
from concourse import mybir
from concourse.bass_utils import run_bass_kernel_spmd

B, A, F_, D = 2048, 256, 64, 128
NCORES = 8
BPC = B // NCORES          # 256 batches per core
BPI = 8                    # batches per iteration
ITERS = BPC // BPI         # 32
PAIRS = BPI // 2           # 4
CHUNKS = BPI * 2           # 16 chunks of 128 atoms

BF16 = mybir.dt.bfloat16
F32 = mybir.dt.float32
U8 = mybir.dt.uint8
I32 = mybir.dt.int32
ALU = mybir.AluOpType
ACTF = mybir.ActivationFunctionType
AXIS = mybir.AxisListType

LN10 = float(np.log(10.0))


def build_body(tc, outs, ins):
    """Tile kernel body. ins/outs: dicts of DRAM APs.

    ins: atoms [ITERS,128,CHUNKS*128] bf16, fragT [ITERS,128,PAIRS*128] bf16,
         idx [ITERS,128,CHUNKS] u8, mask [ITERS,128,PAIRS] bf16
    outs: out [128,1] f32
    """
    nc = tc.nc
    ctx = ExitStack()
    with ctx:
        const = ctx.enter_context(tc.tile_pool(name="const", bufs=1))
        dpool = ctx.enter_context(tc.tile_pool(name="dma", bufs=4))
        work = ctx.enter_context(tc.tile_pool(name="work", bufs=3))
        small = ctx.enter_context(tc.tile_pool(name="small", bufs=3))
        pseg = ctx.enter_context(tc.tile_pool(name="pseg", bufs=2, space="PSUM"))
        ptr = ctx.enter_context(tc.tile_pool(name="ptr", bufs=2, space="PSUM"))
        psim = ctx.enter_context(tc.tile_pool(name="psim", bufs=2, space="PSUM"))

        # ---- one-time constants ----
        iota_i32 = const.tile([128, CHUNKS, 64], I32)
        nc.gpsimd.iota(iota_i32[:], [[0, CHUNKS], [1, 64]], channel_multiplier=0)
        iota_bf = const.tile([128, CHUNKS, 64], BF16)
        nc.gpsimd.tensor_copy(iota_bf[:], iota_i32[:])

        # identity (bf16): ident[p,f] = (p == f)
        irow = const.tile([128, 128], I32)
        nc.gpsimd.iota(irow[:], [[1, 128]], channel_multiplier=0)
        icol = const.tile([128, 1], I32)
        nc.gpsimd.iota(icol[:], [[0, 1]], channel_multiplier=1)
        irow_f = const.tile([128, 128], F32)
        nc.gpsimd.tensor_copy(irow_f[:], irow[:])
        icol_f = const.tile([128, 1], F32)
        nc.gpsimd.tensor_copy(icol_f[:], icol[:])
        ident = const.tile([128, 128], BF16)
        nc.vector.tensor_scalar(ident[:], irow_f[:], icol_f[:], None, op0=ALU.is_equal)

        # accumulation slab: [128, ITERS, PAIRS] f32, fully written each run
        slab = const.tile([128, ITERS, PAIRS], F32)

        ln10_t = const.tile([128, 1], F32)
        nc.gpsimd.memset(ln10_t[:], LN10)

        for it in range(ITERS):
            # ---- DMA in: big contiguous slabs, dispatch spread across
            # engines (SP's DGE-config cost ~600ns/dma was the v1 gate) ----
            atoms_t = dpool.tile([128, CHUNKS, 128], BF16, tag="atoms")
            av = atoms_t[:].rearrange("p c d -> p (c d)")
            nc.gpsimd.dma_start(av[:, 0:512], ins["atoms"][it, :, 0:512])
            nc.gpsimd.dma_start(av[:, 512:1024], ins["atoms"][it, :, 512:1024])
            nc.sync.dma_start(av[:, 1024:1536], ins["atoms"][it, :, 1024:1536])
            nc.sync.dma_start(av[:, 1536:2048], ins["atoms"][it, :, 1536:2048])
            fragT_t = dpool.tile([128, PAIRS, 128], BF16, tag="fragT")
            nc.gpsimd.dma_start(
                fragT_t[:].rearrange("p j d -> p (j d)"), ins["fragT"][it]
            )
            meta_t = dpool.tile([128, 24], U8, tag="meta")
            nc.gpsimd.dma_start(meta_t[:], ins["meta"][it])
            idx_t = meta_t[:, 0:16]
            mask_t = meta_t[:, 16:24].bitcast(BF16)

            # ---- one-hot ----
            idx_bf = small.tile([128, CHUNKS], BF16, tag="idxbf")
            nc.gpsimd.tensor_copy(idx_bf[:], idx_t)
            onehot = work.tile([128, CHUNKS, 64], BF16, tag="onehot")
            idx_bc = idx_bf[:].unsqueeze(2).broadcast_to((128, CHUNKS, 64))
            nc.vector.tensor_tensor(onehot[:], iota_bf[:], idx_bc, op=ALU.is_equal)

            # ---- segment-sum matmuls: psum_seg[64*tw:+64, j, :] += onehot_c.T @ atoms_c
            psum_seg = pseg.tile([128, PAIRS, 128], F32, tag="seg")
            for c in range(CHUNKS):
                bi, h = divmod(c, 2)
                j, tw = divmod(bi, 2)
                nc.tensor.matmul(
                    psum_seg[64 * tw : 64 * tw + 64, j, :],
                    onehot[:, c, :],
                    atoms_t[:, c, :],
                    start=(h == 0),
                    stop=(h == 1),
                )

            # ---- sums -> SBUF bf16 (ACT copy) ----
            sums_s = work.tile([128, PAIRS, 128], BF16, tag="sums")
            nc.scalar.copy(sums_s[:], psum_seg[:])

            # ---- ssq + normalization scale: scale = 10 / max(||sums||, 1e-8)
            sq_s = work.tile([128, PAIRS, 128], BF16, tag="sqs")
            nc.vector.tensor_tensor(sq_s[:], sums_s[:], sums_s[:], op=ALU.mult)
            ssq = small.tile([128, PAIRS], F32, tag="ssq")
            nc.vector.tensor_reduce(ssq[:], sq_s[:], axis=AXIS.X, op=ALU.add)
            ssq_c = small.tile([128, PAIRS], F32, tag="ssqc")
            nc.vector.tensor_scalar(ssq_c[:], ssq[:], 1e-16, None, op0=ALU.max)
            lnssq = small.tile([128, PAIRS], F32, tag="lnssq")
            nc.scalar.activation(lnssq[:], ssq_c[:], ACTF.Ln)
            scale_mn = small.tile([128, PAIRS], F32, tag="scale")
            # exp(-0.5*ln(ssq) + ln(10)) = 10 * rsqrt(ssq)
            nc.scalar.activation(
                scale_mn[:], lnssq[:], ACTF.Exp, bias=ln10_t[:], scale=-0.5
            )

            # ---- D = diag(scale): ident row p has its single 1 at col p ----
            D_t = work.tile([128, PAIRS, 128], BF16, tag="D")
            for j in range(PAIRS):
                nc.vector.tensor_scalar(
                    D_t[:, j, :], ident[:], scale_mn[:, j : j + 1], None, op0=ALU.mult
                )

            # ---- fused transpose+normalize (regular matmul): mnT = sums.T @ D
            psum_tr = ptr.tile([128, PAIRS, 128], F32, tag="tr")
            for j in range(PAIRS):
                nc.tensor.matmul(
                    psum_tr[:, j, :],
                    sums_s[:, j, :],
                    D_t[:, j, :],
                    start=True,
                    stop=True,
                )
            mnT_s = work.tile([128, PAIRS, 128], BF16, tag="mnT")
            nc.scalar.copy(mnT_s[:], psum_tr[:])

            # ---- sims matmuls (rows already normalized & x10) ----
            psum_sims = psim.tile([128, PAIRS, 128], F32, tag="sims")
            for j in range(PAIRS):
                nc.tensor.matmul(
                    psum_sims[:, j, :],
                    mnT_s[:, j, :],
                    fragT_t[:, j, :],
                    start=True,
                    stop=True,
                )

            # ---- exp over all sims (one plain ACT op) ----
            exp_s = work.tile([128, PAIRS, 128], BF16, tag="exps")
            nc.scalar.activation(exp_s[:], psum_sims[:], ACTF.Exp)

            # ---- exp(pos) = diag(exp_s) via strided DMA (off compute engines) ----
            epos = small.tile([128, PAIRS], BF16, tag="epos")
            e_h = exp_s[:, 0, 0]
            diag_ap = bass.AP(
                tensor=e_h.tensor,
                offset=e_h.offset,
                ap=[[PAIRS * 128 + 1, 128], [128, PAIRS]],
            )
            nc.gpsimd.dma_start(epos[:], diag_ap)

            # ---- per-half row sums, select own half ----
            s_half = small.tile([128, PAIRS, 2], F32, tag="shalf")
            exp_v = exp_s[:].rearrange("p j (two g) -> p j two g", two=2)
            nc.vector.tensor_reduce(s_half[:], exp_v, axis=AXIS.X, op=ALU.add)
            s_sel = small.tile([128, PAIRS], F32, tag="ssel")
            nc.vector.tensor_copy(s_sel[0:64, :], s_half[0:64, :, 0])
            nc.vector.tensor_copy(s_sel[64:128, :], s_half[64:128, :, 1])

            # ---- per_frag = ln(s_sel / exp(pos)); masked into slab ----
            inv_ep = small.tile([128, PAIRS], F32, tag="invep")
            nc.vector.reciprocal(inv_ep[:], epos[:])
            ratio = small.tile([128, PAIRS], F32, tag="ratio")
            nc.vector.tensor_tensor(ratio[:], s_sel[:], inv_ep[:], op=ALU.mult)
            perfrag = small.tile([128, PAIRS], F32, tag="perfrag")
            nc.scalar.activation(perfrag[:], ratio[:], ACTF.Ln)
            nc.vector.tensor_tensor(
                slab[:, it, :], perfrag[:], mask_t, op=ALU.mult
            )

        # ---- final reduce + output ----
        outsb = const.tile([128, 1], F32)
        nc.vector.tensor_reduce(outsb[:], slab[:], axis=AXIS.XY, op=ALU.add)
        nc.sync.dma_start(outs["out"], outsb[:])


def prep_inputs(atom_embed, fragment_embed, atom2frag):
    """Host-side layout prep. Returns (in_maps, n_valid)."""
    bf = ml_dtypes.bfloat16
    am = np.asarray(atom_embed, dtype=np.float32)
    fe = np.asarray(fragment_embed, dtype=np.float32)
    af = np.asarray(atom2frag)

    # atoms: [B,A,D] -> [core, it, p, (bi,h), d]
    a6 = am.reshape(NCORES, ITERS, BPI, 2, 128, D)
    atoms_np = np.ascontiguousarray(a6.transpose(0, 1, 4, 2, 3, 5)).reshape(
        NCORES, ITERS, 128, CHUNKS * 128
    ).astype(bf)

    # frag normalized + transposed: [core, it, d, (j, tw*64+g)]
    fen = fe / np.maximum(np.linalg.norm(fe, axis=-1, keepdims=True), 1e-8)
    f6 = fen.reshape(NCORES, ITERS, PAIRS, 2, F_, D)
    fragT_np = np.ascontiguousarray(f6.transpose(0, 1, 5, 2, 3, 4)).reshape(
        NCORES, ITERS, 128, PAIRS * 128
    ).astype(bf)

    # idx: [core, it, p, (bi,h)] u8
    i5 = af.reshape(NCORES, ITERS, BPI, 2, 128)
    idx_np = np.ascontiguousarray(i5.transpose(0, 1, 4, 2, 3)).reshape(
        NCORES, ITERS, 128, CHUNKS
    ).astype(np.uint8)

    # counts/valid on host (index metadata)
    counts = (af[:, :, None] == np.arange(F_)[None, None, :]).sum(axis=1)
    valid = counts > 0
    n_valid = int(valid.sum())
    v6 = valid.reshape(NCORES, ITERS, PAIRS, 2, F_)
    mask_np = np.ascontiguousarray(v6.transpose(0, 1, 3, 4, 2)).reshape(
        NCORES, ITERS, 128, PAIRS
    ).astype(bf)

    # pack idx (16B) + mask-as-bytes (8B) into one small tensor
    meta_np = np.concatenate(
        [idx_np, mask_np.view(np.uint8)], axis=-1
    )  # [NCORES, ITERS, 128, 24]

    in_maps = [
        {
            "atoms": atoms_np[k],
            "fragT": fragT_np[k],
            "meta": meta_np[k],
        }
        for k in range(NCORES)
    ]
    return in_maps, n_valid


_BUILT = None


def build_nc():
    global _BUILT
    if _BUILT is not None:
        return _BUILT
    nc = bacc.Bacc("TRN2", target_bir_lowering=False, debug=False)
    ins = {
        "atoms": nc.dram_tensor(
            "atoms", [ITERS, 128, CHUNKS * 128], BF16, kind="ExternalInput"
        ).ap(),
        "fragT": nc.dram_tensor(
            "fragT", [ITERS, 128, PAIRS * 128], BF16, kind="ExternalInput"
        ).ap(),
        "meta": nc.dram_tensor(
            "meta", [ITERS, 128, 24], U8, kind="ExternalInput"
        ).ap(),
    }
    outs = {"out": nc.dram_tensor("out", [128, 1], F32, kind="ExternalOutput").ap()}
    with tile.TileContext(nc) as tc:
        build_body(tc, outs, ins)
    nc.compile()
    _fix_act_table_loads(nc)
    _BUILT = nc
    return nc


def _fix_act_table_loads(nc):
    """Collapse the Exp<->Ln table-load ping-pong into one load of
    natural_log_exp_and_others (serves Copy/Ln/Exp), saving ~1.3us per load."""
    from concourse.hw_specs import get_activation_tables

    tables = list(get_activation_tables(nc.m.arch).keys())
    target = tables.index("natural_log_exp_and_others")
    kept = False
    for f in nc.m.functions:
        for b in f.blocks:
            keep = []
            for i in b.instructions:
                if isinstance(i, mybir.InstLoadActFuncSet):
                    si = i.sync_info
                    assert si is None or (not si.on_wait and not si.on_update)
                    if kept:
                        continue
                    i.act_func_set_id = target
                    kept = True
                keep.append(i)
            b.instructions[:] = keep


def run_on_hw(in_maps, trace=False, **kw):
    nc = build_nc()
    return run_bass_kernel_spmd(nc, in_maps, list(range(NCORES)), trace=trace, **kw)


def kernel(**inputs) -> np.ndarray:
    in_maps, n_valid = prep_inputs(
        inputs["atom_embed"], inputs["fragment_embed"], inputs["atom2frag"]
    )
    res = run_on_hw(in_maps)
    total = 0.0
    for k in range(NCORES):
        total += float(np.asarray(res.results[k]["out"], dtype=np.float64).sum())
    if n_valid > 0:
        loss = np.float32(total / n_valid)
    else:
        loss = np.float32(0.0)
    return np.array(loss, dtype=np.float32)
